# revision 12
# baseline (speedup 1.0000x reference)
"""Trainium2 Bass kernel for nn_DirectHead (retrieval_knn).

Sharding: images (Bi=256) split 32/core across 8 cores; text side replicated.
Each core computes a [Bt=256, 32] output tile; host concatenates.

Math (equivalent to reference, softmax normalization cancelled):
  i2t[t,i] = num/sqrt(q2)/10 with p_un = exp(20*relu(z*inv_na))*mask,
    z[(a,t),i] = att[t,a].nvis[i], num = sum_a p_un*z, q2 = p^T G p (Gram).
  t2i[t,i] = num2/||y|| with p2_un = exp(20*relu(w*inv_np)),
    w[t,p] = ntext[t].patch[i,p], y = p2_un @ patch_i, num2 = sum_p p2_un*w.
All matmuls in bf16 (fp32 PSUM accumulation); elementwise fp32.
"""
import sys
import numpy as np

for _p in ("/opt/trn_rl_repo",):
    if _p not in sys.path:
        sys.path.insert(0, _p)

import ml_dtypes

BF16 = ml_dtypes.bfloat16

# problem constants (hardcoded per contract)
BT = 256          # text batch
BI = 256          # image batch
NC_ = 8           # cores
IPC = BI // NC_   # images per core = 32
P = 196           # patches per image
PP = 256          # padded patches per image
A = 10            # attributes
V = 768           # input feature dim
D = 1024          # embed dim
KT = V // 128     # 6 contraction tiles
DT = D // 128     # 8 embed-dim tiles
SCALE = 20.0

_CACHE = {}


def _pairs():
    return [(a, a) for a in range(A)] + \
           [(a, b) for a in range(A) for b in range(a + 1, A)]


def _build():
    import concourse.bass as bass
    import concourse.tile as tile
    from concourse import bacc
    import concourse.mybir as mybir
    from concourse.masks import make_identity
    from contextlib import ExitStack

    dt = mybir.dt
    Alu = mybir.AluOpType
    Act = mybir.ActivationFunctionType

    nc = bacc.Bacc("TRN2", target_bir_lowering=False, debug=False,
                   num_devices=NC_)

    # ---- dram I/O (per-core shapes) ----
    d_patchf = nc.dram_tensor("patchf", [V, IPC * P], dt.bfloat16, kind="ExternalInput").ap()
    d_patcht = nc.dram_tensor("patcht", [V, IPC * PP], dt.bfloat16, kind="ExternalInput").ap()
    d_attr = nc.dram_tensor("attr", [V, A * BT], dt.bfloat16, kind="ExternalInput").ap()
    d_text = nc.dram_tensor("text", [V, BT], dt.bfloat16, kind="ExternalInput").ap()
    d_vis = nc.dram_tensor("vis", [V, IPC], dt.bfloat16, kind="ExternalInput").ap()
    d_Wp = nc.dram_tensor("Wp", [V, D], dt.bfloat16, kind="ExternalInput").ap()
    d_Wa = nc.dram_tensor("Wa", [V, D], dt.bfloat16, kind="ExternalInput").ap()
    d_Wt = nc.dram_tensor("Wt", [V, D], dt.bfloat16, kind="ExternalInput").ap()
    d_Wv = nc.dram_tensor("Wv", [V, D], dt.bfloat16, kind="ExternalInput").ap()
    d_mask = nc.dram_tensor("mask01", [128, 2 * A], dt.float32, kind="ExternalInput").ap()
    d_out = nc.dram_tensor("out", [BT, IPC], dt.float32, kind="ExternalOutput").ap()
    d_pt_tm = nc.dram_tensor("pt_tm", [2 * IPC, 128, D], dt.bfloat16).ap()  # internal

    pairs = _pairs()
    pidx = {p: i for i, p in enumerate(pairs)}

    with tile.TileContext(nc) as tc, ExitStack() as top:
        const = top.enter_context(tc.tile_pool(name="const", bufs=1))
        ident = const.tile([128, 128], dt.float32)
        make_identity(nc, ident)
        ones1 = const.tile([1, 128], dt.float32)
        nc.vector.memset(ones1[:], 1.0)
        mask01 = const.tile([128, 2 * A], dt.float32)
        nc.sync.dma_start(mask01[:], d_mask)

        # persistent results
        res = top.enter_context(tc.tile_pool(name="res", bufs=1))
        out_sb = [res.tile([128, IPC], dt.float32, tag=f"out{t}", name=f"out{t}") for t in range(2)]
        G_sb = [res.tile([128, len(pairs)], dt.float32, tag=f"G{t}", name=f"G{t}") for t in range(2)]
        inv_na = res.tile([128, 2, A], dt.float32)
        num2_all = res.tile([128, 2, IPC], dt.float32)
        yn_all = res.tile([128, 2, IPC], dt.float32)
        norms_tm = res.tile([128, 2 * IPC], dt.float32)
        ntextD = [res.tile([128, BT], dt.bfloat16, tag=f"nt{k}", name=f"nt{k}") for k in range(DT)]
        nvisD = [res.tile([128, IPC], dt.bfloat16, tag=f"nv{k}", name=f"nv{k}") for k in range(DT)]

        def load_w(pool, dw):
            tiles = []
            for k in range(KT):
                t = pool.tile([128, D], dt.bfloat16, tag=f"w{k}")
                nc.sync.dma_start(t[:], dw[k * 128:(k + 1) * 128, :])
                tiles.append(t)
            return tiles

        # ---------------- phase 1: attributes ----------------
        with ExitStack() as ph:
            wpool = ph.enter_context(tc.tile_pool(name="wa", bufs=1))
            Wa = load_w(wpool, d_Wa)
            xa = ph.enter_context(tc.tile_pool(name="xa", bufs=1))
            attrT = []
            for k in range(KT):
                t = xa.tile([128, A * BT], dt.bfloat16, tag=f"xa{k}")
                nc.sync.dma_start(t[:], d_attr[k * 128:(k + 1) * 128, :])
                attrT.append(t)
            psum = ph.enter_context(tc.tile_pool(name="ps1", bufs=6, space="PSUM"))
            tmp = ph.enter_context(tc.tile_pool(name="tmp1", bufs=3))
            big = ph.enter_context(tc.tile_pool(name="big1", bufs=1))

            # token-major att embed (20 m-tiles) -> Gram G_sb + inv_na
            gs = ExitStack()
            tmpool = gs.enter_context(tc.tile_pool(name="attm", bufs=1))
            att_tm = []
            for mt in range(2 * A):
                sb = tmpool.tile([128, D], dt.bfloat16, tag=f"tm{mt}")
                for nh in range(2):
                    pt = psum.tile([128, 512], dt.float32, tag="ps")
                    for k in range(KT):
                        nc.tensor.matmul(pt[:], attrT[k][:, mt * 128:(mt + 1) * 128],
                                         Wa[k][:, nh * 512:(nh + 1) * 512],
                                         start=(k == 0), stop=(k == KT - 1))
                    nc.scalar.copy(sb[:, nh * 512:(nh + 1) * 512], pt[:])
                att_tm.append(sb)
            scr = big.tile([128, D], dt.float32, tag="scr")
            scrb = big.tile([128, D], dt.bfloat16, tag="scrb")
            for th in range(2):
                for (a, b) in pairs:
                    nc.vector.tensor_tensor(scrb[:], att_tm[2 * a + th][:],
                                            att_tm[2 * b + th][:], op=Alu.mult)
                    nc.vector.tensor_reduce(G_sb[th][:, pidx[(a, b)]:pidx[(a, b)] + 1],
                                            scrb[:], axis=mybir.AxisListType.X, op=Alu.add)
            for th in range(2):
                t1 = tmp.tile([128, A], dt.float32, tag="t1")
                nc.scalar.activation(t1[:], G_sb[th][:, 0:A], Act.Sqrt)
                nc.vector.reciprocal(inv_na[:, th, :], t1[:])
            gs.close()

            # feature-major att embed attD [d, a*256+t]
            adpool = ph.enter_context(tc.tile_pool(name="attD", bufs=1))
            attD = []
            for m in range(DT):
                sb = adpool.tile([128, A * BT], dt.bfloat16, tag=f"ad{m}")
                for n0 in range(0, A * BT, 512):
                    pt = psum.tile([128, 512], dt.float32, tag="ps")
                    for k in range(KT):
                        nc.tensor.matmul(pt[:], Wa[k][:, m * 128:(m + 1) * 128],
                                         attrT[k][:, n0:n0 + 512],
                                         start=(k == 0), stop=(k == KT - 1))
                    nc.scalar.copy(sb[:, n0:n0 + 512], pt[:])
                attD.append(sb)

            # ---- text + vis norms (needed before z) ----
            wt_pool = ph.enter_context(tc.tile_pool(name="wt", bufs=1))
            Wt = load_w(wt_pool, d_Wt)
            xt = ph.enter_context(tc.tile_pool(name="xt", bufs=1))
            textT = []
            for k in range(KT):
                t = xt.tile([128, BT], dt.bfloat16, tag=f"xt{k}")
                nc.sync.dma_start(t[:], d_text[k * 128:(k + 1) * 128, :])
                textT.append(t)
            # token-major text -> ss -> inv per t
            invnt = tmp.tile([128, 2], dt.float32, tag="invnt")
            for th in range(2):
                sb = big.tile([128, D], dt.float32, tag="ttm")
                for nh in range(2):
                    pt = psum.tile([128, 512], dt.float32, tag="ps")
                    for k in range(KT):
                        nc.tensor.matmul(pt[:], textT[k][:, th * 128:(th + 1) * 128],
                                         Wt[k][:, nh * 512:(nh + 1) * 512],
                                         start=(k == 0), stop=(k == KT - 1))
                    nc.scalar.copy(sb[:, nh * 512:(nh + 1) * 512], pt[:])
                ss = tmp.tile([128, 1], dt.float32, tag="ss")
                nc.scalar.activation(scr[:], sb[:], Act.Square)
                nc.vector.tensor_reduce(ss[:], scr[:], axis=mybir.AxisListType.X, op=Alu.add)
                s1 = tmp.tile([128, 1], dt.float32, tag="s1")
                nc.scalar.activation(s1[:], ss[:], Act.Sqrt)
                nc.vector.reciprocal(invnt[:, th:th + 1], s1[:])
            # transpose [128,2] -> [2,128], broadcast over d-partitions
            bct = tmp.tile([128, BT], dt.float32, tag="bct")
            for th in range(2):
                tpp = psum.tile([1, 128], dt.float32, tag="ps")
                nc.tensor.transpose(tpp[:], invnt[:, th:th + 1], ident[:])
                tps = tmp.tile([1, 128], dt.float32, tag="tps")
                nc.scalar.copy(tps[:], tpp[:])
                bp = psum.tile([128, 128], dt.float32, tag="ps")
                nc.tensor.matmul(bp[:], ones1[:], tps[:], start=True, stop=True)
                nc.scalar.copy(bct[:, th * 128:(th + 1) * 128], bp[:])
            # feature-major text embed, scaled -> ntextD bf16
            for m in range(DT):
                pt = psum.tile([128, 512], dt.float32, tag="ps")
                for k in range(KT):
                    nc.tensor.matmul(pt[:, 0:BT], Wt[k][:, m * 128:(m + 1) * 128],
                                     textT[k][:], start=(k == 0), stop=(k == KT - 1))
                nc.vector.tensor_tensor(ntextD[m][:], pt[:, 0:BT], bct[:], op=Alu.mult)

            # vis
            wv_pool = ph.enter_context(tc.tile_pool(name="wv", bufs=1))
            Wv = load_w(wv_pool, d_Wv)
            xv = ph.enter_context(tc.tile_pool(name="xv", bufs=1))
            visT = []
            for k in range(KT):
                t = xv.tile([128, IPC], dt.bfloat16, tag=f"xv{k}")
                nc.sync.dma_start(t[:], d_vis[k * 128:(k + 1) * 128, :])
                visT.append(t)
            vtm = big.tile([IPC, D], dt.float32, tag="vtm")
            for nh in range(2):
                pt = psum.tile([IPC, 512], dt.float32, tag="ps")
                for k in range(KT):
                    nc.tensor.matmul(pt[:], visT[k][:], Wv[k][:, nh * 512:(nh + 1) * 512],
                                     start=(k == 0), stop=(k == KT - 1))
                nc.scalar.copy(vtm[:, nh * 512:(nh + 1) * 512], pt[:])
            ssv = tmp.tile([IPC, 1], dt.float32, tag="ssv")
            scrv = big.tile([IPC, D], dt.float32, tag="scrv")
            nc.scalar.activation(scrv[:], vtm[:], Act.Square)
            nc.vector.tensor_reduce(ssv[:], scrv[:], axis=mybir.AxisListType.X, op=Alu.add)
            sv1 = tmp.tile([IPC, 1], dt.float32, tag="sv1")
            nc.scalar.activation(sv1[:], ssv[:], Act.Sqrt)
            rv = tmp.tile([IPC, 1], dt.float32, tag="rv")
            nc.vector.reciprocal(rv[:], sv1[:])
            tpv = psum.tile([1, IPC], dt.float32, tag="ps")
            nc.tensor.transpose(tpv[:], rv[:], ident[0:IPC, 0:IPC])
            tpvs = tmp.tile([1, IPC], dt.float32, tag="tpvs")
            nc.scalar.copy(tpvs[:], tpv[:])
            bv = psum.tile([128, IPC], dt.float32, tag="ps")
            nc.tensor.matmul(bv[:], ones1[:], tpvs[:], start=True, stop=True)
            bvs = tmp.tile([128, IPC], dt.float32, tag="bvs")
            nc.scalar.copy(bvs[:], bv[:])
            for m in range(DT):
                pt = psum.tile([128, IPC], dt.float32, tag="ps")
                for k in range(KT):
                    nc.tensor.matmul(pt[:], Wv[k][:, m * 128:(m + 1) * 128], visT[k][:],
                                     start=(k == 0), stop=(k == KT - 1))
                nc.vector.tensor_tensor(nvisD[m][:], pt[:], bvs[:], op=Alu.mult)

            # ---------------- i2t ----------------
            zpool = ph.enter_context(tc.tile_pool(name="zp", bufs=1))
            ppool = ph.enter_context(tc.tile_pool(name="pp", bufs=1))
            z_sb, p_sb = {}, {}
            for j in range(2 * A):
                zp = psum.tile([128, IPC], dt.float32, tag="ps")
                for k in range(DT):
                    nc.tensor.matmul(zp[:], attD[k][:, j * 128:(j + 1) * 128],
                                     nvisD[k][:], start=(k == 0), stop=(k == DT - 1))
                z = zpool.tile([128, IPC], dt.float32, tag=f"z{j}")
                nc.scalar.copy(z[:], zp[:])
                a, th = j // 2, j % 2
                s = tmp.tile([128, IPC], dt.float32, tag="sA")
                nc.vector.tensor_scalar(out=s[:], in0=zp[:],
                                        scalar1=inv_na[:, th, a:a + 1],
                                        scalar2=0.0, op0=Alu.mult, op1=Alu.max)
                p = ppool.tile([128, IPC], dt.float32, tag=f"p{j}")
                nc.scalar.activation(p[:], s[:], Act.Exp, scale=SCALE,
                                     bias=mask01[:, j:j + 1])
                z_sb[j], p_sb[j] = z, p
            for th in range(2):
                num = tmp.tile([128, IPC], dt.float32, tag="num")
                q2 = tmp.tile([128, IPC], dt.float32, tag="q2")
                t2 = tmp.tile([128, IPC], dt.float32, tag="t2")
                for a in range(A):
                    j = 2 * a + th
                    if a == 0:
                        nc.vector.tensor_tensor(num[:], p_sb[j][:], z_sb[j][:], op=Alu.mult)
                    else:
                        nc.vector.scalar_tensor_tensor(out=t2[:], in0=p_sb[j][:], scalar=1.0,
                                                       in1=z_sb[j][:], op0=Alu.mult, op1=Alu.mult)
                        nc.vector.tensor_tensor(num[:], num[:], t2[:], op=Alu.add)
                first = True
                for (a, b) in pairs:
                    gcol = G_sb[th][:, pidx[(a, b)]:pidx[(a, b)] + 1]
                    dst = q2 if first else t2
                    nc.vector.scalar_tensor_tensor(out=dst[:], in0=p_sb[2 * a + th][:],
                                                   scalar=gcol, in1=p_sb[2 * b + th][:],
                                                   op0=Alu.mult, op1=Alu.mult)
                    if not first:
                        nc.vector.tensor_tensor(q2[:], q2[:], t2[:], op=Alu.add)
                        if a != b:
                            nc.vector.tensor_tensor(q2[:], q2[:], t2[:], op=Alu.add)
                    first = False
                sq = tmp.tile([128, IPC], dt.float32, tag="sqq")
                nc.scalar.activation(sq[:], q2[:], Act.Sqrt)
                rc = tmp.tile([128, IPC], dt.float32, tag="rcq")
                nc.vector.reciprocal(rc[:], sq[:])
                nc.vector.scalar_tensor_tensor(out=out_sb[th][:], in0=num[:], scalar=0.1,
                                               in1=rc[:], op0=Alu.mult, op1=Alu.mult)

        # ---------------- phase 2: patch embeds ----------------
        pdpool = top.enter_context(tc.tile_pool(name="patchD", bufs=1))
        patchD = [pdpool.tile([128, IPC * P], dt.bfloat16, tag=f"pd{m}", name=f"pd{m}")
                  for m in range(DT)]
        with ExitStack() as ph:
            wpool = ph.enter_context(tc.tile_pool(name="wp", bufs=1))
            Wp = load_w(wpool, d_Wp)
            psum = ph.enter_context(tc.tile_pool(name="ps2", bufs=6, space="PSUM"))
            tmp = ph.enter_context(tc.tile_pool(name="tmp2", bufs=3))
            xs = ph.enter_context(tc.tile_pool(name="xs", bufs=4))
            # feature-major
            NTOT = IPC * P
            n0 = 0
            while n0 < NTOT:
                nw = min(512, NTOT - n0)
                xk = []
                for k in range(KT):
                    x = xs.tile([128, 512], dt.bfloat16, tag=f"xs{k}")
                    nc.sync.dma_start(x[:, 0:nw], d_patchf[k * 128:(k + 1) * 128, n0:n0 + nw])
                    xk.append(x)
                for m in range(DT):
                    pt = psum.tile([128, 512], dt.float32, tag="ps")
                    for k in range(KT):
                        nc.tensor.matmul(pt[:, 0:nw], Wp[k][:, m * 128:(m + 1) * 128],
                                         xk[k][:, 0:nw], start=(k == 0), stop=(k == KT - 1))
                    nc.scalar.copy(patchD[m][:, n0:n0 + nw], pt[:, 0:nw])
                n0 += nw
            # token-major -> norms + DRAM
            scr = tmp.tile([128, D], dt.float32, tag="scr2")
            CH = 16  # m-tiles per DMA load group (2048-col chunks, 4KB rows)
            xgrp = {}
            for mt in range(2 * IPC):
                if mt % CH == 0:
                    xgrp = {}
                    for k in range(KT):
                        xg = xs.tile([128, CH * 128], dt.bfloat16, tag=f"xg{k}", bufs=2)
                        nc.sync.dma_start(xg[:], d_patcht[k * 128:(k + 1) * 128,
                                                          mt * 128:(mt + CH) * 128])
                        xgrp[k] = xg
                xk = [xgrp[k][:, (mt % CH) * 128:(mt % CH + 1) * 128] for k in range(KT)]
                sb = tmp.tile([128, D], dt.bfloat16, tag="ptm")
                for nh in range(2):
                    pt = psum.tile([128, 512], dt.float32, tag="ps")
                    for k in range(KT):
                        nc.tensor.matmul(pt[:], xk[k], Wp[k][:, nh * 512:(nh + 1) * 512],
                                         start=(k == 0), stop=(k == KT - 1))
                    nc.scalar.copy(sb[:, nh * 512:(nh + 1) * 512], pt[:])
                nc.scalar.activation(scr[:], sb[:], Act.Square,
                                     accum_out=norms_tm[:, mt:mt + 1])
                nc.sync.dma_start(d_pt_tm[mt], sb[:])

        # ---------------- phase 3: t2i per image ----------------
        with ExitStack() as ph:
            psA = ph.enter_context(tc.tile_pool(name="psA", bufs=1, space="PSUM"))
            psW = ph.enter_context(tc.tile_pool(name="psW", bufs=3, space="PSUM"))
            psY = ph.enter_context(tc.tile_pool(name="psY", bufs=2, space="PSUM"))
            tmp = ph.enter_context(tc.tile_pool(name="tmp3", bufs=4))
            ptp = ph.enter_context(tc.tile_pool(name="ptp", bufs=4))
            for i in range(IPC):
                # inv-norm broadcast for this image's patches
                bc = psA.tile([128, P], dt.float32, tag="bc")
                for kp in range(2):
                    tp = psA.tile([1, 128], dt.float32, tag="tp")
                    nc.tensor.transpose(tp[:], norms_tm[:, 2 * i + kp:2 * i + kp + 1], ident[:])
                    tps = tmp.tile([1, 128], dt.float32, tag="tps")
                    nc.scalar.copy(tps[:], tp[:])
                    nw = 128 if kp == 0 else P - 128
                    nc.tensor.matmul(bc[:, kp * 128:kp * 128 + nw], ones1[:],
                                     tps[:, 0:nw], start=True, stop=True)
                sqn = tmp.tile([128, P], dt.float32, tag="sqn")
                nc.scalar.activation(sqn[:], bc[:], Act.Sqrt)
                invp = tmp.tile([128, P], dt.float32, tag="invp")
                nc.vector.reciprocal(invp[:], sqn[:])
                pt_i = []
                for kp in range(2):
                    t = ptp.tile([128, D], dt.bfloat16, tag=f"pti{kp}")
                    nc.sync.dma_start(t[:], d_pt_tm[2 * i + kp])
                    pt_i.append(t)
                for th in range(2):
                    w = psW.tile([128, P], dt.float32, tag="w")
                    for k in range(DT):
                        nc.tensor.matmul(w[:], ntextD[k][:, th * 128:(th + 1) * 128],
                                         patchD[k][:, i * P:(i + 1) * P],
                                         start=(k == 0), stop=(k == DT - 1))
                    s2 = tmp.tile([128, P], dt.float32, tag="s2")
                    nc.vector.scalar_tensor_tensor(out=s2[:], in0=w[:], scalar=0.0,
                                                   in1=invp[:], op0=Alu.max, op1=Alu.mult)
                    p2 = tmp.tile([128, PP], dt.float32, tag="p2")
                    nc.vector.memset(p2[:, P:PP], 0.0)
                    nc.scalar.activation(p2[:, 0:P], s2[:], Act.Exp, scale=SCALE)
                    scrp = tmp.tile([128, P], dt.float32, tag="scrp")
                    nc.vector.tensor_tensor(scrp[:], p2[:, 0:P], w[:], op=Alu.mult)
                    nc.vector.tensor_reduce(num2_all[:, th, i:i + 1], scrp[:],
                                            axis=mybir.AxisListType.X, op=Alu.add)
                    p2T = []
                    for kp in range(2):
                        tpp2 = psA.tile([128, 128], dt.float32, tag="tpp2")
                        nc.tensor.transpose(tpp2[:], p2[:, kp * 128:(kp + 1) * 128], ident[:])
                        t = tmp.tile([128, 128], dt.bfloat16, tag=f"p2T{kp}")
                        nc.scalar.copy(t[:], tpp2[:])
                        p2T.append(t)
                    nsss = []
                    for nh in range(2):
                        yp = psY.tile([128, 512], dt.float32, tag="yp")
                        for kp in range(2):
                            nc.tensor.matmul(yp[:], p2T[kp][:],
                                             pt_i[kp][:, nh * 512:(nh + 1) * 512],
                                             start=(kp == 0), stop=(kp == 1))
                        scy = tmp.tile([128, 512], dt.float32, tag="scy")
                        nss = tmp.tile([128, 1], dt.float32, tag=f"nss{nh}")
                        nc.scalar.activation(scy[:], yp[:], Act.Square,
                                             accum_out=nss[:])
                        nsss.append(nss)
                    nc.vector.tensor_tensor(yn_all[:, th, i:i + 1], nsss[0][:],
                                            nsss[1][:], op=Alu.add)

        # batched t2i finalize: one sqrt/recip over all 64 (th, i) norms
        with ExitStack() as ph:
            ftmp = ph.enter_context(tc.tile_pool(name="fin", bufs=1))
            sqy = ftmp.tile([128, 2, IPC], dt.float32)
            nc.scalar.activation(sqy[:], yn_all[:], Act.Sqrt)
            rcy = ftmp.tile([128, 2, IPC], dt.float32)
            nc.vector.reciprocal(rcy[:], sqy[:])
            t2i = ftmp.tile([128, 2, IPC], dt.float32)
            nc.vector.tensor_tensor(t2i[:], num2_all[:], rcy[:], op=Alu.mult)
            for th in range(2):
                nc.vector.tensor_tensor(out_sb[th][:], out_sb[th][:], t2i[:, th, :],
                                        op=Alu.add)
                nc.sync.dma_start(d_out[th * 128:(th + 1) * 128, :], out_sb[th][:])

    nc.compile()
    return nc


def _prep(inputs):
    vf = np.asarray(inputs["visual_feature"], np.float32)
    tf = np.asarray(inputs["textual_feature"], np.float32)
    af = np.asarray(inputs["attribute_feature"], np.float32)
    an = np.asarray(inputs["att_nums"]).astype(np.int64)
    Wp = np.asarray(inputs["Wp"], np.float32)
    Wa = np.asarray(inputs["Wa"], np.float32)
    Wt = np.asarray(inputs["Wt"], np.float32)
    Wv = np.asarray(inputs["Wv"], np.float32)

    textT = np.ascontiguousarray(tf.T).astype(BF16)                       # [768,256]
    attrT = np.ascontiguousarray(af.transpose(1, 0, 2).reshape(A * BT, V).T).astype(BF16)
    WTs = {n: np.ascontiguousarray(W.T).astype(BF16)
           for n, W in (("Wp", Wp), ("Wa", Wa), ("Wt", Wt), ("Wv", Wv))}
    # mask01 [128, 20]: exp bias, col j=(a, th) -> 0.0 if valid else -50
    m = (np.arange(A)[None, :] < an[:, None])                             # [256,10]
    mask01 = np.empty((128, 2 * A), np.float32)
    for a in range(A):
        for th in range(2):
            mask01[:, 2 * a + th] = np.where(m[th * 128:(th + 1) * 128, a],
                                             0.0, -50.0)

    maps = []
    for c in range(NC_):
        sl = slice(c * IPC, (c + 1) * IPC)
        pat = vf[sl, 1:, :]                                               # [32,196,768]
        patf = np.ascontiguousarray(pat.reshape(IPC * P, V).T).astype(BF16)
        patp = np.zeros((IPC, PP, V), np.float32)
        patp[:, :P, :] = pat
        patt = np.ascontiguousarray(patp.reshape(IPC * PP, V).T).astype(BF16)
        visT = np.ascontiguousarray(vf[sl, 0, :].T).astype(BF16)
        maps.append({
            "patchf": patf, "patcht": patt, "attr": attrT, "text": textT,
            "vis": visT, "Wp": WTs["Wp"], "Wa": WTs["Wa"], "Wt": WTs["Wt"],
            "Wv": WTs["Wv"], "mask01": mask01,
        })
    return maps


def _run(inputs, trace=False):
    from concourse.bass_utils import run_bass_kernel_spmd
    if "nc" not in _CACHE:
        _CACHE["nc"] = _build()
    maps = _prep(inputs)
    res = run_bass_kernel_spmd(_CACHE["nc"], maps, list(range(NC_)), trace=trace)
    out = np.concatenate([res.results[c]["out"] for c in range(NC_)], axis=1)
    return out.astype(np.float32), res


def kernel(**inputs):
    out, _ = _run(inputs, trace=False)
    return out



# revision 13
# speedup vs baseline: 1.4431x; 1.4431x over previous
"""Trainium2 Bass kernel for nn_DirectHead (retrieval_knn).

Sharding: images (Bi=256) split 32/core across 8 cores; text side replicated.
Each core computes a [Bt=256, 32] output tile; host concatenates.

Math (equivalent to reference, softmax normalization cancelled):
  i2t[t,i] = num/sqrt(q2)/10 with p_un = exp(20*relu(z*inv_na))*mask,
    z[(a,t),i] = att[t,a].nvis[i], num = sum_a p_un*z, q2 = p^T G p (Gram).
  t2i[t,i] = num2/||y|| with p2_un = exp(20*relu(w*inv_np)),
    w[t,p] = ntext[t].patch[i,p], y = p2_un @ patch_i, num2 = sum_p p2_un*w.
All matmuls in bf16 (fp32 PSUM accumulation); elementwise fp32.
"""
import sys
import numpy as np

for _p in ("/opt/trn_rl_repo",):
    if _p not in sys.path:
        sys.path.insert(0, _p)

import ml_dtypes

BF16 = ml_dtypes.bfloat16

# problem constants (hardcoded per contract)
BT = 256          # text batch
BI = 256          # image batch
NC_ = 8           # cores
IPC = BI // NC_   # images per core = 32
P = 196           # patches per image
PP = 256          # padded patches per image
A = 10            # attributes
V = 768           # input feature dim
D = 1024          # embed dim
KT = V // 128     # 6 contraction tiles
DT = D // 128     # 8 embed-dim tiles
SCALE = 20.0

_CACHE = {}


def _pairs():
    return [(a, a) for a in range(A)] + \
           [(a, b) for a in range(A) for b in range(a + 1, A)]


def _build():
    import concourse.bass as bass
    import concourse.tile as tile
    from concourse import bacc
    import concourse.mybir as mybir
    from concourse.masks import make_identity
    from contextlib import ExitStack

    dt = mybir.dt
    Alu = mybir.AluOpType
    Act = mybir.ActivationFunctionType

    nc = bacc.Bacc("TRN2", target_bir_lowering=False, debug=False,
                   num_devices=NC_)

    # ---- dram I/O (per-core shapes) ----
    d_patchf = nc.dram_tensor("patchf", [V, IPC * P], dt.bfloat16, kind="ExternalInput").ap()
    d_patcht = nc.dram_tensor("patcht", [V, IPC * PP], dt.bfloat16, kind="ExternalInput").ap()
    d_attr = nc.dram_tensor("attr", [V, A * BT], dt.bfloat16, kind="ExternalInput").ap()
    d_text = nc.dram_tensor("text", [V, BT], dt.bfloat16, kind="ExternalInput").ap()
    d_vis = nc.dram_tensor("vis", [V, IPC], dt.bfloat16, kind="ExternalInput").ap()
    d_Wp = nc.dram_tensor("Wp", [V, D], dt.bfloat16, kind="ExternalInput").ap()
    d_Wa = nc.dram_tensor("Wa", [V, D], dt.bfloat16, kind="ExternalInput").ap()
    d_Wt = nc.dram_tensor("Wt", [V, D], dt.bfloat16, kind="ExternalInput").ap()
    d_Wv = nc.dram_tensor("Wv", [V, D], dt.bfloat16, kind="ExternalInput").ap()
    d_mask = nc.dram_tensor("mask01", [128, 2 * A], dt.float32, kind="ExternalInput").ap()
    d_out = nc.dram_tensor("out", [BT, IPC], dt.float32, kind="ExternalOutput").ap()
    d_pt_tm = nc.dram_tensor("pt_tm", [2 * IPC, 128, D], dt.bfloat16).ap()  # internal

    pairs = _pairs()
    pidx = {p: i for i, p in enumerate(pairs)}

    with tile.TileContext(nc) as tc, ExitStack() as top:
        const = top.enter_context(tc.tile_pool(name="const", bufs=1))
        ident = const.tile([128, 128], dt.float32)
        make_identity(nc, ident)
        ones1 = const.tile([1, 128], dt.float32)
        nc.vector.memset(ones1[:], 1.0)
        mask01 = const.tile([128, 2 * A], dt.float32)
        nc.sync.dma_start(mask01[:], d_mask)

        # persistent results
        res = top.enter_context(tc.tile_pool(name="res", bufs=1))
        out_sb = [res.tile([128, IPC], dt.float32, tag=f"out{t}", name=f"out{t}") for t in range(2)]
        G_sb = [res.tile([128, len(pairs)], dt.float32, tag=f"G{t}", name=f"G{t}") for t in range(2)]
        inv_na = res.tile([128, 2, A], dt.float32)
        num2_all = res.tile([128, 2, IPC], dt.float32)
        yn_all = res.tile([128, 2, IPC], dt.float32)
        norms_tm = res.tile([128, 2 * IPC], dt.float32)
        ntextD = [res.tile([128, BT], dt.bfloat16, tag=f"nt{k}", name=f"nt{k}") for k in range(DT)]
        nvisD = [res.tile([128, IPC], dt.bfloat16, tag=f"nv{k}", name=f"nv{k}") for k in range(DT)]

        def load_w(pool, dw):
            tiles = []
            for k in range(KT):
                t = pool.tile([128, D], dt.bfloat16, tag=f"w{k}")
                nc.sync.dma_start(t[:], dw[k * 128:(k + 1) * 128, :])
                tiles.append(t)
            return tiles

        # ---------------- phase 1: attributes ----------------
        with ExitStack() as ph:
            wpool = ph.enter_context(tc.tile_pool(name="wa", bufs=1))
            Wa = load_w(wpool, d_Wa)
            xa = ph.enter_context(tc.tile_pool(name="xa", bufs=1))
            attrT = []
            for k in range(KT):
                t = xa.tile([128, A * BT], dt.bfloat16, tag=f"xa{k}")
                nc.sync.dma_start(t[:], d_attr[k * 128:(k + 1) * 128, :])
                attrT.append(t)
            psum = ph.enter_context(tc.tile_pool(name="ps1", bufs=6, space="PSUM"))
            tmp = ph.enter_context(tc.tile_pool(name="tmp1", bufs=3))
            big = ph.enter_context(tc.tile_pool(name="big1", bufs=1))

            # token-major att embed (20 m-tiles) -> Gram G_sb + inv_na
            gs = ExitStack()
            tmpool = gs.enter_context(tc.tile_pool(name="attm", bufs=1))
            att_tm = []
            for mt in range(2 * A):
                sb = tmpool.tile([128, D], dt.bfloat16, tag=f"tm{mt}")
                for nh in range(2):
                    pt = psum.tile([128, 512], dt.float32, tag="ps")
                    for k in range(KT):
                        nc.tensor.matmul(pt[:], attrT[k][:, mt * 128:(mt + 1) * 128],
                                         Wa[k][:, nh * 512:(nh + 1) * 512],
                                         start=(k == 0), stop=(k == KT - 1))
                    nc.scalar.copy(sb[:, nh * 512:(nh + 1) * 512], pt[:])
                att_tm.append(sb)
            scr = big.tile([128, D], dt.float32, tag="scr")
            scrb = big.tile([128, D], dt.bfloat16, tag="scrb")
            for th in range(2):
                for (a, b) in pairs:
                    nc.vector.tensor_tensor(scrb[:], att_tm[2 * a + th][:],
                                            att_tm[2 * b + th][:], op=Alu.mult)
                    nc.vector.tensor_reduce(G_sb[th][:, pidx[(a, b)]:pidx[(a, b)] + 1],
                                            scrb[:], axis=mybir.AxisListType.X, op=Alu.add)
            for th in range(2):
                t1 = tmp.tile([128, A], dt.float32, tag="t1")
                nc.scalar.activation(t1[:], G_sb[th][:, 0:A], Act.Sqrt)
                nc.vector.reciprocal(inv_na[:, th, :], t1[:])
            gs.close()

            # feature-major att embed attD [d, a*256+t]
            adpool = ph.enter_context(tc.tile_pool(name="attD", bufs=1))
            attD = []
            for m in range(DT):
                sb = adpool.tile([128, A * BT], dt.bfloat16, tag=f"ad{m}")
                for n0 in range(0, A * BT, 512):
                    pt = psum.tile([128, 512], dt.float32, tag="ps")
                    for k in range(KT):
                        nc.tensor.matmul(pt[:], Wa[k][:, m * 128:(m + 1) * 128],
                                         attrT[k][:, n0:n0 + 512],
                                         start=(k == 0), stop=(k == KT - 1))
                    nc.scalar.copy(sb[:, n0:n0 + 512], pt[:])
                attD.append(sb)

            # ---- text + vis norms (needed before z) ----
            wt_pool = ph.enter_context(tc.tile_pool(name="wt", bufs=1))
            Wt = load_w(wt_pool, d_Wt)
            xt = ph.enter_context(tc.tile_pool(name="xt", bufs=1))
            textT = []
            for k in range(KT):
                t = xt.tile([128, BT], dt.bfloat16, tag=f"xt{k}")
                nc.sync.dma_start(t[:], d_text[k * 128:(k + 1) * 128, :])
                textT.append(t)
            # token-major text -> ss -> inv per t
            invnt = tmp.tile([128, 2], dt.float32, tag="invnt")
            for th in range(2):
                sb = big.tile([128, D], dt.float32, tag="ttm")
                for nh in range(2):
                    pt = psum.tile([128, 512], dt.float32, tag="ps")
                    for k in range(KT):
                        nc.tensor.matmul(pt[:], textT[k][:, th * 128:(th + 1) * 128],
                                         Wt[k][:, nh * 512:(nh + 1) * 512],
                                         start=(k == 0), stop=(k == KT - 1))
                    nc.scalar.copy(sb[:, nh * 512:(nh + 1) * 512], pt[:])
                ss = tmp.tile([128, 1], dt.float32, tag="ss")
                nc.scalar.activation(scr[:], sb[:], Act.Square)
                nc.vector.tensor_reduce(ss[:], scr[:], axis=mybir.AxisListType.X, op=Alu.add)
                s1 = tmp.tile([128, 1], dt.float32, tag="s1")
                nc.scalar.activation(s1[:], ss[:], Act.Sqrt)
                nc.vector.reciprocal(invnt[:, th:th + 1], s1[:])
            # transpose [128,2] -> [2,128], broadcast over d-partitions
            bct = tmp.tile([128, BT], dt.float32, tag="bct")
            for th in range(2):
                tpp = psum.tile([1, 128], dt.float32, tag="ps")
                nc.tensor.transpose(tpp[:], invnt[:, th:th + 1], ident[:])
                tps = tmp.tile([1, 128], dt.float32, tag="tps")
                nc.scalar.copy(tps[:], tpp[:])
                bp = psum.tile([128, 128], dt.float32, tag="ps")
                nc.tensor.matmul(bp[:], ones1[:], tps[:], start=True, stop=True)
                nc.scalar.copy(bct[:, th * 128:(th + 1) * 128], bp[:])
            # feature-major text embed, scaled -> ntextD bf16
            for m in range(DT):
                pt = psum.tile([128, 512], dt.float32, tag="ps")
                for k in range(KT):
                    nc.tensor.matmul(pt[:, 0:BT], Wt[k][:, m * 128:(m + 1) * 128],
                                     textT[k][:], start=(k == 0), stop=(k == KT - 1))
                nc.vector.tensor_tensor(ntextD[m][:], pt[:, 0:BT], bct[:], op=Alu.mult)

            # vis
            wv_pool = ph.enter_context(tc.tile_pool(name="wv", bufs=1))
            Wv = load_w(wv_pool, d_Wv)
            xv = ph.enter_context(tc.tile_pool(name="xv", bufs=1))
            visT = []
            for k in range(KT):
                t = xv.tile([128, IPC], dt.bfloat16, tag=f"xv{k}")
                nc.sync.dma_start(t[:], d_vis[k * 128:(k + 1) * 128, :])
                visT.append(t)
            vtm = big.tile([IPC, D], dt.float32, tag="vtm")
            for nh in range(2):
                pt = psum.tile([IPC, 512], dt.float32, tag="ps")
                for k in range(KT):
                    nc.tensor.matmul(pt[:], visT[k][:], Wv[k][:, nh * 512:(nh + 1) * 512],
                                     start=(k == 0), stop=(k == KT - 1))
                nc.scalar.copy(vtm[:, nh * 512:(nh + 1) * 512], pt[:])
            ssv = tmp.tile([IPC, 1], dt.float32, tag="ssv")
            scrv = big.tile([IPC, D], dt.float32, tag="scrv")
            nc.scalar.activation(scrv[:], vtm[:], Act.Square)
            nc.vector.tensor_reduce(ssv[:], scrv[:], axis=mybir.AxisListType.X, op=Alu.add)
            sv1 = tmp.tile([IPC, 1], dt.float32, tag="sv1")
            nc.scalar.activation(sv1[:], ssv[:], Act.Sqrt)
            rv = tmp.tile([IPC, 1], dt.float32, tag="rv")
            nc.vector.reciprocal(rv[:], sv1[:])
            tpv = psum.tile([1, IPC], dt.float32, tag="ps")
            nc.tensor.transpose(tpv[:], rv[:], ident[0:IPC, 0:IPC])
            tpvs = tmp.tile([1, IPC], dt.float32, tag="tpvs")
            nc.scalar.copy(tpvs[:], tpv[:])
            bv = psum.tile([128, IPC], dt.float32, tag="ps")
            nc.tensor.matmul(bv[:], ones1[:], tpvs[:], start=True, stop=True)
            bvs = tmp.tile([128, IPC], dt.float32, tag="bvs")
            nc.scalar.copy(bvs[:], bv[:])
            for m in range(DT):
                pt = psum.tile([128, IPC], dt.float32, tag="ps")
                for k in range(KT):
                    nc.tensor.matmul(pt[:], Wv[k][:, m * 128:(m + 1) * 128], visT[k][:],
                                     start=(k == 0), stop=(k == KT - 1))
                nc.vector.tensor_tensor(nvisD[m][:], pt[:], bvs[:], op=Alu.mult)

            # ---------------- i2t ----------------
            zpool = ph.enter_context(tc.tile_pool(name="zp", bufs=1))
            ppool = ph.enter_context(tc.tile_pool(name="pp", bufs=1))
            z_sb, p_sb = {}, {}
            for j in range(2 * A):
                zp = psum.tile([128, IPC], dt.float32, tag="ps")
                for k in range(DT):
                    nc.tensor.matmul(zp[:], attD[k][:, j * 128:(j + 1) * 128],
                                     nvisD[k][:], start=(k == 0), stop=(k == DT - 1))
                z = zpool.tile([128, IPC], dt.float32, tag=f"z{j}")
                nc.scalar.copy(z[:], zp[:])
                a, th = j // 2, j % 2
                s = tmp.tile([128, IPC], dt.float32, tag="sA")
                nc.vector.tensor_scalar(out=s[:], in0=zp[:],
                                        scalar1=inv_na[:, th, a:a + 1],
                                        scalar2=0.0, op0=Alu.mult, op1=Alu.max)
                p = ppool.tile([128, IPC], dt.float32, tag=f"p{j}")
                nc.scalar.activation(p[:], s[:], Act.Exp, scale=SCALE,
                                     bias=mask01[:, j:j + 1])
                z_sb[j], p_sb[j] = z, p
            for th in range(2):
                num = tmp.tile([128, IPC], dt.float32, tag="num")
                q2 = tmp.tile([128, IPC], dt.float32, tag="q2")
                t2 = tmp.tile([128, IPC], dt.float32, tag="t2")
                for a in range(A):
                    j = 2 * a + th
                    if a == 0:
                        nc.vector.tensor_tensor(num[:], p_sb[j][:], z_sb[j][:], op=Alu.mult)
                    else:
                        nc.vector.scalar_tensor_tensor(out=t2[:], in0=p_sb[j][:], scalar=1.0,
                                                       in1=z_sb[j][:], op0=Alu.mult, op1=Alu.mult)
                        nc.vector.tensor_tensor(num[:], num[:], t2[:], op=Alu.add)
                first = True
                for (a, b) in pairs:
                    gcol = G_sb[th][:, pidx[(a, b)]:pidx[(a, b)] + 1]
                    dst = q2 if first else t2
                    nc.vector.scalar_tensor_tensor(out=dst[:], in0=p_sb[2 * a + th][:],
                                                   scalar=gcol, in1=p_sb[2 * b + th][:],
                                                   op0=Alu.mult, op1=Alu.mult)
                    if not first:
                        nc.vector.tensor_tensor(q2[:], q2[:], t2[:], op=Alu.add)
                        if a != b:
                            nc.vector.tensor_tensor(q2[:], q2[:], t2[:], op=Alu.add)
                    first = False
                sq = tmp.tile([128, IPC], dt.float32, tag="sqq")
                nc.scalar.activation(sq[:], q2[:], Act.Sqrt)
                rc = tmp.tile([128, IPC], dt.float32, tag="rcq")
                nc.vector.reciprocal(rc[:], sq[:])
                nc.vector.scalar_tensor_tensor(out=out_sb[th][:], in0=num[:], scalar=0.1,
                                               in1=rc[:], op0=Alu.mult, op1=Alu.mult)

        # ---------------- phase 2: patch embeds ----------------
        pdpool = top.enter_context(tc.tile_pool(name="patchD", bufs=1))
        patchD = [pdpool.tile([128, IPC * P], dt.bfloat16, tag=f"pd{m}", name=f"pd{m}")
                  for m in range(DT)]
        with ExitStack() as ph:
            wpool = ph.enter_context(tc.tile_pool(name="wp", bufs=1))
            Wp = load_w(wpool, d_Wp)
            psum = ph.enter_context(tc.tile_pool(name="ps2", bufs=6, space="PSUM"))
            tmp = ph.enter_context(tc.tile_pool(name="tmp2", bufs=3))
            xs = ph.enter_context(tc.tile_pool(name="xs", bufs=4))
            # feature-major
            NTOT = IPC * P
            n0 = 0
            while n0 < NTOT:
                nw = min(512, NTOT - n0)
                xk = []
                for k in range(KT):
                    x = xs.tile([128, 512], dt.bfloat16, tag=f"xs{k}")
                    nc.sync.dma_start(x[:, 0:nw], d_patchf[k * 128:(k + 1) * 128, n0:n0 + nw])
                    xk.append(x)
                for m in range(DT):
                    pt = psum.tile([128, 512], dt.float32, tag="ps")
                    for k in range(KT):
                        nc.tensor.matmul(pt[:, 0:nw], Wp[k][:, m * 128:(m + 1) * 128],
                                         xk[k][:, 0:nw], start=(k == 0), stop=(k == KT - 1))
                    nc.scalar.copy(patchD[m][:, n0:n0 + nw], pt[:, 0:nw])
                n0 += nw
            # token-major -> norms + DRAM
            scr = tmp.tile([128, D], dt.float32, tag="scr2")
            CH = 16  # m-tiles per DMA load group (2048-col chunks, 4KB rows)
            xgrp = {}
            for mt in range(2 * IPC):
                if mt % CH == 0:
                    xgrp = {}
                    for k in range(KT):
                        xg = xs.tile([128, CH * 128], dt.bfloat16, tag=f"xg{k}", bufs=2)
                        nc.sync.dma_start(xg[:], d_patcht[k * 128:(k + 1) * 128,
                                                          mt * 128:(mt + CH) * 128])
                        xgrp[k] = xg
                xk = [xgrp[k][:, (mt % CH) * 128:(mt % CH + 1) * 128] for k in range(KT)]
                sb = tmp.tile([128, D], dt.bfloat16, tag="ptm")
                for nh in range(2):
                    pt = psum.tile([128, 512], dt.float32, tag="ps")
                    for k in range(KT):
                        nc.tensor.matmul(pt[:], xk[k], Wp[k][:, nh * 512:(nh + 1) * 512],
                                         start=(k == 0), stop=(k == KT - 1))
                    nc.scalar.copy(sb[:, nh * 512:(nh + 1) * 512], pt[:])
                nc.scalar.activation(scr[:], sb[:], Act.Square,
                                     accum_out=norms_tm[:, mt:mt + 1])
                nc.sync.dma_start(d_pt_tm[mt], sb[:])

        # ---------------- phase 3: t2i per image ----------------
        with ExitStack() as ph:
            invtp = ph.enter_context(tc.tile_pool(name="invt", bufs=1))
            sq_nt = invtp.tile([128, 2 * IPC], dt.float32)
            nc.scalar.activation(sq_nt[:], norms_tm[:], Act.Sqrt)
            inv_nt = invtp.tile([128, 2 * IPC], dt.float32)
            nc.vector.reciprocal(inv_nt[:], sq_nt[:])
            psA = ph.enter_context(tc.tile_pool(name="psA", bufs=1, space="PSUM"))
            psW = ph.enter_context(tc.tile_pool(name="psW", bufs=3, space="PSUM"))
            psY = ph.enter_context(tc.tile_pool(name="psY", bufs=2, space="PSUM"))
            tmp = ph.enter_context(tc.tile_pool(name="tmp3", bufs=4))
            ptp = ph.enter_context(tc.tile_pool(name="ptp", bufs=4))
            for i in range(IPC):
                # inv-norm broadcast for this image's patches
                bcp = psA.tile([128, P], dt.float32, tag="bc")
                for kp in range(2):
                    tp = psA.tile([1, 128], dt.float32, tag="tp")
                    nc.tensor.transpose(tp[:], inv_nt[:, 2 * i + kp:2 * i + kp + 1],
                                        ident[:])
                    tps = tmp.tile([1, 128], dt.float32, tag="tps")
                    nc.scalar.copy(tps[:], tp[:])
                    nw = 128 if kp == 0 else P - 128
                    nc.tensor.matmul(bcp[:, kp * 128:kp * 128 + nw], ones1[:],
                                     tps[:, 0:nw], start=True, stop=True)
                invp = tmp.tile([128, P], dt.float32, tag="invp")
                nc.scalar.copy(invp[:], bcp[:])
                pt_i = []
                for kp in range(2):
                    t = ptp.tile([128, D], dt.bfloat16, tag=f"pti{kp}")
                    nc.sync.dma_start(t[:], d_pt_tm[2 * i + kp])
                    pt_i.append(t)
                for th in range(2):
                    w = psW.tile([128, P], dt.float32, tag="w")
                    for k in range(DT):
                        nc.tensor.matmul(w[:], ntextD[k][:, th * 128:(th + 1) * 128],
                                         patchD[k][:, i * P:(i + 1) * P],
                                         start=(k == 0), stop=(k == DT - 1))
                    s2 = tmp.tile([128, P], dt.float32, tag="s2")
                    nc.vector.scalar_tensor_tensor(out=s2[:], in0=w[:], scalar=0.0,
                                                   in1=invp[:], op0=Alu.max, op1=Alu.mult)
                    p2 = tmp.tile([128, PP], dt.float32, tag="p2")
                    nc.vector.memset(p2[:, P:PP], 0.0)
                    nc.scalar.activation(p2[:, 0:P], s2[:], Act.Exp, scale=SCALE)
                    scrp = tmp.tile([128, P], dt.float32, tag="scrp")
                    nc.vector.tensor_tensor(scrp[:], p2[:, 0:P], w[:], op=Alu.mult)
                    nc.vector.tensor_reduce(num2_all[:, th, i:i + 1], scrp[:],
                                            axis=mybir.AxisListType.X, op=Alu.add)
                    p2T = []
                    for kp in range(2):
                        tpp2 = psA.tile([128, 128], dt.float32, tag="tpp2")
                        nc.tensor.transpose(tpp2[:], p2[:, kp * 128:(kp + 1) * 128], ident[:])
                        t = tmp.tile([128, 128], dt.bfloat16, tag=f"p2T{kp}")
                        nc.scalar.copy(t[:], tpp2[:])
                        p2T.append(t)
                    nsss = []
                    for nh in range(2):
                        yp = psY.tile([128, 512], dt.float32, tag="yp")
                        for kp in range(2):
                            nc.tensor.matmul(yp[:], p2T[kp][:],
                                             pt_i[kp][:, nh * 512:(nh + 1) * 512],
                                             start=(kp == 0), stop=(kp == 1))
                        scy = tmp.tile([128, 512], dt.float32, tag="scy")
                        nss = tmp.tile([128, 1], dt.float32, tag=f"nss{nh}")
                        nc.scalar.activation(scy[:], yp[:], Act.Square,
                                             accum_out=nss[:])
                        nsss.append(nss)
                    nc.vector.tensor_tensor(yn_all[:, th, i:i + 1], nsss[0][:],
                                            nsss[1][:], op=Alu.add)

        # batched t2i finalize: one sqrt/recip over all 64 (th, i) norms
        with ExitStack() as ph:
            ftmp = ph.enter_context(tc.tile_pool(name="fin", bufs=1))
            sqy = ftmp.tile([128, 2, IPC], dt.float32)
            nc.scalar.activation(sqy[:], yn_all[:], Act.Sqrt)
            rcy = ftmp.tile([128, 2, IPC], dt.float32)
            nc.vector.reciprocal(rcy[:], sqy[:])
            t2i = ftmp.tile([128, 2, IPC], dt.float32)
            nc.vector.tensor_tensor(t2i[:], num2_all[:], rcy[:], op=Alu.mult)
            for th in range(2):
                nc.vector.tensor_tensor(out_sb[th][:], out_sb[th][:], t2i[:, th, :],
                                        op=Alu.add)
                nc.sync.dma_start(d_out[th * 128:(th + 1) * 128, :], out_sb[th][:])

    nc.compile()
    return nc


def _prep(inputs):
    vf = np.asarray(inputs["visual_feature"], np.float32)
    tf = np.asarray(inputs["textual_feature"], np.float32)
    af = np.asarray(inputs["attribute_feature"], np.float32)
    an = np.asarray(inputs["att_nums"]).astype(np.int64)
    Wp = np.asarray(inputs["Wp"], np.float32)
    Wa = np.asarray(inputs["Wa"], np.float32)
    Wt = np.asarray(inputs["Wt"], np.float32)
    Wv = np.asarray(inputs["Wv"], np.float32)

    textT = np.ascontiguousarray(tf.T).astype(BF16)                       # [768,256]
    attrT = np.ascontiguousarray(af.transpose(1, 0, 2).reshape(A * BT, V).T).astype(BF16)
    WTs = {n: np.ascontiguousarray(W.T).astype(BF16)
           for n, W in (("Wp", Wp), ("Wa", Wa), ("Wt", Wt), ("Wv", Wv))}
    # mask01 [128, 20]: exp bias, col j=(a, th) -> 0.0 if valid else -50
    m = (np.arange(A)[None, :] < an[:, None])                             # [256,10]
    mask01 = np.empty((128, 2 * A), np.float32)
    for a in range(A):
        for th in range(2):
            mask01[:, 2 * a + th] = np.where(m[th * 128:(th + 1) * 128, a],
                                             0.0, -50.0)

    maps = []
    for c in range(NC_):
        sl = slice(c * IPC, (c + 1) * IPC)
        pat = vf[sl, 1:, :]                                               # [32,196,768]
        patf = np.ascontiguousarray(pat.reshape(IPC * P, V).T).astype(BF16)
        patp = np.zeros((IPC, PP, V), np.float32)
        patp[:, :P, :] = pat
        patt = np.ascontiguousarray(patp.reshape(IPC * PP, V).T).astype(BF16)
        visT = np.ascontiguousarray(vf[sl, 0, :].T).astype(BF16)
        maps.append({
            "patchf": patf, "patcht": patt, "attr": attrT, "text": textT,
            "vis": visT, "Wp": WTs["Wp"], "Wa": WTs["Wa"], "Wt": WTs["Wt"],
            "Wv": WTs["Wv"], "mask01": mask01,
        })
    return maps


def _run(inputs, trace=False):
    from concourse.bass_utils import run_bass_kernel_spmd
    if "nc" not in _CACHE:
        _CACHE["nc"] = _build()
    maps = _prep(inputs)
    res = run_bass_kernel_spmd(_CACHE["nc"], maps, list(range(NC_)), trace=trace)
    out = np.concatenate([res.results[c]["out"] for c in range(NC_)], axis=1)
    return out.astype(np.float32), res


def kernel(**inputs):
    out, _ = _run(inputs, trace=False)
    return out

